# revision 1
# baseline (speedup 1.0000x reference)
"""Causal self-attention for (2, 2048, 1024), 16 heads, on 8 trn2 cores.

Sharding: batch x head-group. Core c handles batch b = c // 4 and heads
[4*(c%4), 4*(c%4)+4). Each core computes q/k/v projections for its 4 heads
from the (host-pre-transposed) hidden states of its batch, runs causal
attention per head fully in transposed layout, applies its slice of the
output projection, and returns a [2048, 1024] partial. The host sums the 4
partials per batch and adds the output bias.

All matmuls run in fp32r (TF32-like, 1 cycle/row at N>=256 vs 4 for fp32).
Attention uses the transposed-scores formulation so softmax needs no
max-subtraction (inputs are well-scaled) and the row-sum comes free from an
appended ones-column in the ctx matmul stationary operand.
"""

import sys

sys.path.insert(0, "/opt/trn_rl_repo")

import ml_dtypes
import numpy as np

import concourse.bass as bass
from concourse.bass import _add_dep_helper
import concourse.mybir as mybir
import concourse.tile as tile
from concourse.vector_clock import ScopedClock

B, S, H, NH, HD = 2, 2048, 1024, 16, 64
NCORES = 8
HPC = 4          # heads per core
CHUNK = 512      # i-chunk width (PSUM bank)
NIT = S // 128   # 16 i-tiles (128 queries each)
NIC = S // CHUNK # 4 i-chunks
KT = H // 128    # 8 contraction tiles for projections
SCALE = 1.0 / np.sqrt(HD)

f32 = mybir.dt.float32
f32r = mybir.dt.float32r
bf16 = mybir.dt.bfloat16
EXP = mybir.ActivationFunctionType.Exp
MUL = mybir.AluOpType.mult
ADD = mybir.AluOpType.add


class _TC(tile.TileContext):
    """TileContext whose tail drain carries no sem waits: this walrus build
    rejects instructions with more than one sync-wait command, so the waits
    are emitted as individual wait_ge instructions instead."""

    def _drain_and_barrier(self, tick_clock, wait_clock):
        nc = self.nc
        carrier = nc.sync.nop()
        wait_clock.add_sem_waits(
            carrier.ins, ScopedClock({None: tick_clock.global_clock})
        )
        si = carrier.ins.sync_info
        waits = list(si.on_wait) if si and si.on_wait else []
        si.on_wait = []
        assert self.sems is not None
        id2handle = {h.num: h for h in self.sems.allocated().values()}
        for w in waits:
            nc.sync.wait_ge(id2handle[w.id], w.wait_value)
        nc.sync.drain()
        nc.all_engine_barrier()
        popped = nc._tile_sem_poison_stack.pop()
        assert popped is self._sem_poison
        nc.clear_and_free_semaphores(list(self.sems.allocated().values()))
        nc.all_engine_barrier()


_waitfix_ctr = [0]


def _split_multiwaits(nc):
    """Hoist all-but-one sync wait off every instruction into standalone
    single-wait EventSemaphore instructions (same engine, same position)."""
    for f in nc.m.functions:
        for bb in f.blocks:
            out = []
            changed = False
            for inst in bb.instructions:
                si = inst.sync_info
                waits = list(si.on_wait) if si and si.on_wait else []
                if len(waits) > 1:
                    changed = True
                    for w in waits[:-1]:
                        _waitfix_ctr[0] += 1
                        ev = mybir.InstEventSemaphore(
                            name=f"I-waitfix-{_waitfix_ctr[0]}",
                            engine=inst.engine,
                            ins=[],
                            outs=[],
                            sync_info=mybir.SyncInfo(on_wait=[w], on_update=[]),
                        )
                        nc.register_instruction(ev)
                        out.append(ev)
                    si.on_wait = waits[-1:]
                out.append(inst)
            if changed:
                bb.instructions = out


def _build_program():
    nc = bass.Bass("TRN2", target_bir_lowering=False, debug=False,
                   num_devices=NCORES)

    xt = nc.dram_tensor("xt", [H, S], bf16, kind="ExternalInput")
    wq = nc.dram_tensor("wq", [H, HPC * HD], bf16, kind="ExternalInput")
    wk = nc.dram_tensor("wk", [H, HPC * HD], bf16, kind="ExternalInput")
    wv = nc.dram_tensor("wv", [H, HPC * HD], bf16, kind="ExternalInput")
    wo = nc.dram_tensor("wo", [HPC * HD, H], bf16, kind="ExternalInput")
    bqkv = nc.dram_tensor("bqkv", [128, 6], f32, kind="ExternalInput")
    ones64 = nc.dram_tensor("ones64", [1, 64], f32, kind="ExternalInput")
    mask = nc.dram_tensor("mask", [128, 128], bf16, kind="ExternalInput")
    ones16 = nc.dram_tensor("ones16", [128, NIT], bf16, kind="ExternalInput")
    ident2 = nc.dram_tensor("ident2", [128, 64], bf16, kind="ExternalInput")
    outp = nc.dram_tensor("outp", [S, H], f32, kind="ExternalOutput")

    last_pe = [None]

    def _mm(inst):
        if last_pe[0] is not None:
            _add_dep_helper(inst.ins, last_pe[0].ins, sync=False,
                            reason="pe emission order")
        last_pe[0] = inst
        return inst

    with _TC(nc) as tc:
        with (
            tc.tile_pool(name="const", bufs=1) as constp,
            tc.tile_pool(name="qk", bufs=1) as qkp,
            tc.tile_pool(name="vj", bufs=1) as vjp,
            tc.tile_pool(name="ctxT2", bufs=1) as ctxT2p,
        ):
            ones64_sb = constp.tile([1, 64], f32r)
            nc.sync.dma_start(ones64_sb[:], ones64.ap().bitcast(f32r))
            mask_sb = constp.tile([128, 128], bf16)
            nc.sync.dma_start(mask_sb[:], mask.ap())
            bqkv_sb = constp.tile([128, 6], f32)
            nc.sync.dma_start(bqkv_sb[:], bqkv.ap())
            ident2_sb = constp.tile([128, 64], bf16)
            nc.sync.dma_start(ident2_sb[:], ident2.ap())

            qt_sb = qkp.tile([128, 2, S], bf16, tag="qt")
            kt_sb = qkp.tile([128, 2, S], bf16, tag="kt")
            # v'[j, d] per head with a ones column appended (col 64), packed
            # as [j-in-tile, head, j-tile, d|1]
            vj_sb = vjp.tile([128, HPC, NIT, HD + 1], bf16)
            for h in range(HPC):
                nc.sync.dma_start(vj_sb[:, h, :, HD:HD + 1],
                                  ones16.ap())

            # normalized ctx^T, packed [d-in-pair, pair, i]
            ctxT2_sb = ctxT2p.tile([128, 2, S], bf16)


            # ---- phase A: projections -------------------------------------
            with (
                tc.tile_pool(name="xtp", bufs=1) as xtp,
                tc.tile_pool(name="wqkv", bufs=1) as wqkvp,
                tc.tile_pool(name="vt", bufs=1) as vtp,
                tc.tile_pool(name="mm", bufs=4, space="PSUM") as mmp,
                tc.tile_pool(name="vtr", bufs=2, space="PSUM") as vtrp,
            ):
                xt_sb = xtp.tile([128, KT, S], bf16)
                for t in range(KT):
                    nc.sync.dma_start(
                        xt_sb[:, t, :], xt.ap()[t * 128:(t + 1) * 128, :]
                    )
                # per-k-tile weight DMAs: contiguous 64KB blocks spread over
                # queues, so the first V matmul isn't gated on one ~22us
                # monolithic strided transfer (same per-region dep pattern
                # already used for xt above)
                w_sb = {}
                for name, wten in (("v", wv), ("k", wk), ("q", wq)):
                    w_sb[name] = wqkvp.tile([128, KT, HPC * HD], bf16, tag=name, name=f"w_{name}")
                    for t in range(KT):
                        nc.sync.dma_start(
                            w_sb[name][:, t, :],
                            wten.ap()[t * 128:(t + 1) * 128, :],
                        )

                vt_sb = vtp.tile([128, 2, S], bf16)
                dst = {"q": qt_sb, "k": kt_sb, "v": vt_sb}
                boff = {"q": 0, "k": 2, "v": 4}
                for name in ("v", "k", "q"):
                    for dt_ in range(2):  # d-tile: 128 output rows each
                        pss = [mmp.tile([128, CHUNK], f32, tag="mm",
                                        name=f"mm_{name}{dt_}{sc}")
                               for sc in range(NIC)]
                        for t in range(KT):
                            for sc in range(NIC):
                                _mm(nc.tensor.matmul(
                                    pss[sc][:],
                                    w_sb[name][:, t, dt_ * 128:(dt_ + 1) * 128],
                                    xt_sb[:, t, sc * CHUNK:(sc + 1) * CHUNK],
                                    start=(t == 0),
                                    stop=(t == KT - 1),
                                ))
                        for sc in range(NIC):
                            nc.vector.tensor_scalar(
                                out=dst[name][:, dt_, sc * CHUNK:(sc + 1) * CHUNK],
                                in0=pss[sc][:],
                                scalar1=bqkv_sb[:, boff[name] + dt_:boff[name] + dt_ + 1],
                                scalar2=None,
                                op0=ADD,
                            )
                    if name == "v":
                        # build v' = vT.T per head, j-tile by j-tile
                        for h in range(HPC):
                            vrow = vt_sb[(h % 2) * 64:(h % 2) * 64 + 64, h // 2, :]
                            for jt in range(NIT):
                                tp = vtrp.tile([128, HD], bf16, tag="vtr")
                                bp = (h % 2) * 64
                                _mm(nc.tensor.transpose(
                                    tp[:],
                                    vrow[:, jt * 128:(jt + 1) * 128],
                                    ident2_sb[bp:bp + 64, :],
                                ))
                                nc.vector.tensor_copy(
                                    vj_sb[:, h, jt, 0:HD], tp[:]
                                )

            # ---- phase B: attention ---------------------------------------
            with tc.tile_pool(name="ctxu", bufs=1) as ctxup:
                # unnormalized ctx (+rowsum in row 64) per (head, i-chunk)
                ctxu_sb = ctxup.tile([HD + 1, HPC * NIC, CHUNK], f32r)
                attn = tc.tile_pool(name="sc", bufs=2, space="PSUM")
                scp = attn.__enter__()
                ctxpool = tc.tile_pool(name="ctx", bufs=4, space="PSUM")
                ctxp = ctxpool.__enter__()
                ptpool = tc.tile_pool(name="pt", bufs=6)
                ptp = ptpool.__enter__()
                for h in range(HPC):
                    qrow = qt_sb[(h % 2) * 64:(h % 2) * 64 + 64, h // 2, :]
                    krow = kt_sb[(h % 2) * 64:(h % 2) * 64 + 64, h // 2, :]
                    ctx_ps = [ctxp.tile([HD + 1, CHUNK], f32, tag="ctx",
                                        name=f"ctx_{h}_{ic}")
                              for ic in range(NIC)]
                    pt_tiles = {}

                    def emit_scores(jt):
                        ic0 = jt // 4
                        for pair in range(2):
                            lo = max(ic0, pair * 2)
                            hi = pair * 2 + 2
                            if lo >= hi:
                                continue
                            sc_ps = scp.tile([128, 2, CHUNK], f32, tag="sc")
                            pt_sb = ptp.tile([128, 2, CHUNK], bf16, tag="pt")
                            for ic in range(lo, hi):
                                off = max(0, jt * 128 - ic * CHUNK)
                                _mm(nc.tensor.matmul(
                                    sc_ps[:, ic - pair * 2, off:CHUNK],
                                    krow[:, jt * 128:(jt + 1) * 128],
                                    qrow[:, ic * CHUNK + off:(ic + 1) * CHUNK],
                                    start=True,
                                    stop=True,
                                ))
                            off0 = max(0, jt * 128 - lo * CHUNK)
                            flat_lo = (lo - pair * 2) * CHUNK + off0
                            scf = sc_ps[:].rearrange("p a b -> p (a b)")
                            ptf = pt_sb[:].rearrange("p a b -> p (a b)")
                            nc.scalar.activation(
                                ptf[:, flat_lo:2 * CHUNK],
                                scf[:, flat_lo:2 * CHUNK],
                                EXP,
                                scale=float(SCALE),
                            )
                            if pair * 2 <= ic0 < hi:  # diagonal block
                                nc.vector.tensor_tensor(
                                    out=ptf[:, flat_lo:flat_lo + 128],
                                    in0=ptf[:, flat_lo:flat_lo + 128],
                                    in1=mask_sb[:],
                                    op=MUL,
                                )
                            for ic in range(lo, hi):
                                pt_tiles[(jt, ic)] = (pt_sb, ic - pair * 2)

                    def emit_ctx(jt):
                        for ic in range(jt // 4, NIC):
                            off = max(0, jt * 128 - ic * CHUNK)
                            width = CHUNK - off
                            pt_sb, sub = pt_tiles.pop((jt, ic))
                            _mm(nc.tensor.matmul(
                                ctx_ps[ic][:, off:off + width],
                                vj_sb[:, h, jt, :],
                                pt_sb[:, sub, off:off + width],
                                start=(jt == 0),
                                stop=(jt == 4 * ic + 3),
                            ))

                    emit_scores(0)
                    for jt in range(1, NIT):
                        emit_scores(jt)
                        emit_ctx(jt - 1)
                    emit_ctx(NIT - 1)
                    for ic in range(NIC):
                        nc.vector.tensor_copy(
                            ctxu_sb[:, h * NIC + ic, :], ctx_ps[ic][:]
                        )
                ptpool.__exit__(None, None, None)
                ctxpool.__exit__(None, None, None)
                attn.__exit__(None, None, None)
                # ---- phase B2: normalize via rowsum broadcast + divide ----
                bcpool = tc.tile_pool(name="bc", bufs=3, space="PSUM")
                bcp = bcpool.__enter__()
                bcsbpool = tc.tile_pool(name="bcsb", bufs=3)
                bcsb = bcsbpool.__enter__()
                DIV = mybir.AluOpType.divide
                for h in range(HPC):
                    for ic in range(NIC):
                        rsrow = bcsb.tile([1, CHUNK], f32r, tag="rsrow")
                        with nc.allow_low_precision(
                                reason="rowsum reciprocal rounded to f32r"):
                            nc.vector.reciprocal(
                                rsrow[:], ctxu_sb[HD:HD + 1, h * NIC + ic, :]
                            )
                        bc = bcp.tile([HD, CHUNK], f32, tag="bc")
                        _mm(nc.tensor.matmul(
                            bc[:],
                            ones64_sb[:],
                            rsrow[:],
                            start=True,
                            stop=True,
                        ))
                        nc.vector.tensor_tensor(
                            out=ctxT2_sb[(h % 2) * 64:(h % 2) * 64 + 64, h // 2,
                                         ic * CHUNK:(ic + 1) * CHUNK],
                            in0=ctxu_sb[0:HD, h * NIC + ic, :].bitcast(f32),
                            in1=bc[:],
                            op=MUL,
                        )
                bcsbpool.__exit__(None, None, None)
                bcpool.__exit__(None, None, None)

            # ---- phase C: output projection -------------------------------
            with (
                tc.tile_pool(name="om", bufs=4, space="PSUM") as omp,
                tc.tile_pool(name="osb", bufs=10) as osbp,
                tc.tile_pool(name="wop", bufs=1) as wop,
            ):
                wo_sb = wop.tile([128, 2, H], bf16)
                for p in range(2):
                    nc.sync.dma_start(
                        wo_sb[:, p, :],
                        wo.ap()[p * 128:(p + 1) * 128, :],
                    )
                for it in range(NIT):
                    pso = [omp.tile([128, CHUNK], f32, tag="om",
                                    name=f"om_{it}_{nck}")
                           for nck in range(H // CHUNK)]
                    for p in range(2):
                        for nck in range(H // CHUNK):
                            _mm(nc.tensor.matmul(
                                pso[nck][:],
                                ctxT2_sb[:, p, it * 128:(it + 1) * 128],
                                wo_sb[:, p, nck * CHUNK:(nck + 1) * CHUNK],
                                start=(p == 0),
                                stop=(p == 1),
                            ))
                    for nck in range(H // CHUNK):
                        osb = osbp.tile([128, CHUNK], f32, tag="osb")
                        nc.vector.tensor_copy(osb[:], pso[nck][:])
                        nc.sync.dma_start(
                            outp.ap()[it * 128:(it + 1) * 128,
                                      nck * CHUNK:(nck + 1) * CHUNK],
                            osb[:],
                        )

    _split_multiwaits(nc)
    return nc


_nc_cache = None


def _get_program():
    global _nc_cache
    if _nc_cache is None:
        _nc_cache = _build_program()
    return _nc_cache


def kernel(hidden_states, Wq, bq, Wk, bk, Wv, bv, Wo, bo):
    from concourse.bass_utils import run_bass_kernel_spmd

    hidden_states = np.asarray(hidden_states, dtype=np.float32)
    Wq, bq = np.asarray(Wq, np.float32), np.asarray(bq, np.float32)
    Wk, bk = np.asarray(Wk, np.float32), np.asarray(bk, np.float32)
    Wv, bv = np.asarray(Wv, np.float32), np.asarray(bv, np.float32)
    Wo, bo = np.asarray(Wo, np.float32), np.asarray(bo, np.float32)

    ones64 = np.ones((1, 64), np.float32)
    ident2 = np.tile(np.eye(64, dtype=ml_dtypes.bfloat16), (2, 1)).copy()
    # mask[j, i_local] = 1 where query i >= key j inside a diagonal block
    mask = np.tril(np.ones((128, 128), ml_dtypes.bfloat16)).T.copy()
    ones16 = np.ones((128, NIT), ml_dtypes.bfloat16)

    in_maps = []
    for c in range(NCORES):
        b = c // (NCORES // B)
        hg = c % (NCORES // B)
        hsel = slice(hg * HPC * HD, (hg + 1) * HPC * HD)
        xt = np.ascontiguousarray(hidden_states[b].T).astype(ml_dtypes.bfloat16)
        bq_c = bq[hsel].reshape(2, 128).T.copy()
        bk_c = bk[hsel].reshape(2, 128).T.copy()
        bv_c = bv[hsel].reshape(2, 128).T.copy()
        bqkv_c = np.concatenate([bq_c, bk_c, bv_c], axis=1)
        in_maps.append({
            "xt": xt,
            "wq": np.ascontiguousarray(Wq[:, hsel]).astype(ml_dtypes.bfloat16),
            "wk": np.ascontiguousarray(Wk[:, hsel]).astype(ml_dtypes.bfloat16),
            "wv": np.ascontiguousarray(Wv[:, hsel]).astype(ml_dtypes.bfloat16),
            "wo": np.ascontiguousarray(Wo[hsel, :]).astype(ml_dtypes.bfloat16),
            "bqkv": bqkv_c,
            "ones64": ones64,
            "ident2": ident2,
            "mask": mask,
            "ones16": ones16,
        })

    res = run_bass_kernel_spmd(_get_program(), in_maps, list(range(NCORES)))
    out = np.zeros((B, S, H), np.float32)
    for c in range(NCORES):
        out[c // (NCORES // B)] += res.results[c]["outp"]
    out += bo[None, None, :]
    return out

